# revision 2
# baseline (speedup 1.0000x reference)
"""Trainium2 Bass kernel for the ADI diffusion layer — band-stencil version.

Math: each ADI step applies three tridiagonal solves (x, y, x) per channel.
The tridiagonal matrices here are diagonally dominant with off-diagonal
ratio ~5e-4 (coeff = smooth(alpha)*dt/dx^2 with dt=1e-3), so each solve
operator T^-1 is numerically a *banded* matrix: its entries decay by ~5e-4
per off-diagonal.  Truncating to the tridiagonal band of T^-1 gives a
3-point stencil per sweep with relative error ~1e-6/sweep (measured 2.5e-5
end-to-end vs the fp64 reference — far inside the 2e-2 gate).

Two more reductions:
  * adjacent half-step x-sweeps at step boundaries share the same operator
    and are merged (band of T^-2), cutting 30 sweeps to 21;
  * the diagonal of each stencil is absorbed into a per-element running
    scale (host-precomputed), leaving out = v + Wm.shift(v) + Wp.shift(v):
    only TWO weighted-multiply passes per sweep on the DVE at 1 elem/cycle
    (the scan formulation needed 2 passes at 2 cyc/elem — feedback-limited).

Device dataflow per sweep (data always stays in x-layout: partition=h,
free=(c, batch, w); y-sweeps never transpose):
  x-sweep: DVE: T1,T2 = Wm'*V, Wp'*V (one k-merged rank-4 instr/channel);
           PE:  POUT = I*V (+) I*T1 shifted +1 (+) I*T2 shifted -1  (PSUM);
           ACT: V = copy(POUT).
  y-sweep: PE:  SD = ShiftDown*V, SU = ShiftUp*V (partition shifts, PSUM);
           DVE: T1,T2 = Wm*SD, Wp*SU (reads PSUM);
           PE:  POUT = I*V (+) I*T1 (+) I*T2;   ACT: V = copy(POUT).
Per-core work is B/8 = 4 batch planes x 3 channels; pure data parallel
across the 8 cores (coefficients replicated).
"""
import numpy as np

import concourse.bass as bass
from concourse import mybir
from concourse.bass_utils import run_bass_kernel_spmd

# ---- problem constants (hardcoded per contract) ----
B, C, S = 32, 3, 128
NCORES = 8
BL = B // NCORES            # 4 batch planes per core
DT, DX, DY = 0.001, 1.0, 1.0
NUM_STEPS = 10
EPS = 1e-6
NSW = 2 * NUM_STEPS + 1     # 21 sweeps: x, (y, x)*10 with merged double-x
W = BL * S                  # 512 packed free cols per channel
CW = C * W                  # 1536
MCOLS = NSW * 2 * C * S     # 16128 weight cols

F32 = mybir.dt.float32
MUL = mybir.AluOpType.mult


# ---------------- host-side stencil precompute ----------------

def _smooth(c):
    p = np.pad(c, [(0, 0)] * (c.ndim - 1) + [(1, 1)], mode="edge")
    return (p[..., :-2] + p[..., 1:-1] + p[..., 2:]) / 3.0


def _thomas64(a, b, c, d):
    n = d.shape[-1]
    cs = np.empty_like(d)
    ds = np.empty_like(d)
    den = b[..., 0] + EPS
    cs[..., 0] = c[..., 0] / den
    ds[..., 0] = d[..., 0] / den
    for i in range(1, n):
        den = b[..., i] - a[..., i] * cs[..., i - 1] + EPS
        cs[..., i] = c[..., i] / den
        ds[..., i] = (d[..., i] - a[..., i] * ds[..., i - 1]) / den
    x = np.empty_like(d)
    x[..., -1] = ds[..., -1]
    for i in range(n - 2, -1, -1):
        x[..., i] = ds[..., i] - cs[..., i] * x[..., i + 1]
    return x


def _band_of_solve(coef, dt, dx, power):
    """(Wm, W0, Wp) of the solve operator (or its square) along the last
    axis, extracted with 3-comb solves in fp64. coef: (C,S,S)."""
    coeff = _smooth(coef) * dt / (dx ** 2)
    a = -coeff.copy()
    b = 1.0 + 2.0 * coeff
    b[..., 0] = 1.0 + coeff[..., 0]
    b[..., -1] = 1.0 + coeff[..., -1]
    c = -coeff.copy()
    shp = coef.shape
    n = shp[-1]
    Wm = np.zeros(shp)
    W0 = np.zeros(shp)
    Wp = np.zeros(shp)
    idx = np.arange(n)
    for k in range(3):
        comb = np.zeros(n)
        comb[k::3] = 1.0
        X = _thomas64(a, b, c, np.broadcast_to(comb, shp).copy())
        if power == 2:
            X = _thomas64(a, b, c, X)
        sel0 = (idx % 3) == k
        W0[..., sel0] = X[..., sel0]
        selm = ((idx - 1) % 3) == k
        selm[0] = False
        Wm[..., selm] = X[..., selm]
        selp = ((idx + 1) % 3) == k
        selp[-1] = False
        Wp[..., selp] = X[..., selp]
    return Wm, W0, Wp


def _build_packed(alpha_base, beta_base, alpha_tc, beta_tc):
    """Returns (mults (128, MCOLS) f32, aux (128, 768) f32).
    mults per sweep s: cols [768s,768s+384) Wm-block (c-major 128 cols),
    [768s+384,768s+768) Wp-block; x-sweep blocks pre-shifted to align with
    the source element; y-sweep blocks in x-orientation (h, w) aligned to
    the PE-shifted source.  aux: [ID | ShiftDown | ShiftUp | SF]."""
    f8 = np.float64
    ab, bb = alpha_base.astype(f8), beta_base.astype(f8)
    atc, btc = alpha_tc.astype(f8), beta_tc.astype(f8)
    clamp = lambda base, tc, t: np.maximum(base + tc * t, EPS)

    sw = [("x", 0.0, DT / 2, 1)]
    for k in range(NUM_STEPS):
        t = k * DT
        sw.append(("y", t + DT / 2, DT, 1))
        sw.append(("x", t + DT, DT / 2, 2 if k < NUM_STEPS - 1 else 1))

    mults = np.zeros((128, MCOLS), dtype=np.float32)
    S_run = np.ones((C, S, S), dtype=f8)     # x-orientation (c, h, w)
    for s, (which, tt, dt_, power) in enumerate(sw):
        if which == "x":
            coef = clamp(ab, atc, tt)
            Sv = S_run
        else:
            coef = np.swapaxes(clamp(bb, btc, tt), -1, -2)
            Sv = np.swapaxes(S_run, -1, -2)
        Wm, W0, Wp = _band_of_solve(coef, dt_, DX if which == "x" else DY,
                                    power)
        Sp = W0 * Sv
        Wmt = np.zeros_like(Wm)
        Wpt = np.zeros_like(Wp)
        Wmt[..., 1:] = Wm[..., 1:] * Sv[..., :-1] / Sp[..., 1:]
        Wpt[..., :-1] = Wp[..., :-1] * Sv[..., 1:] / Sp[..., :-1]
        S_run = Sp if which == "x" else np.swapaxes(Sp, -1, -2)
        if which == "x":
            # pre-shift: t1[w] = Wmt[w+1]*v[w]  /  t2[w] = Wpt[w-1]*v[w]
            Wm_dev = np.zeros_like(Wmt)
            Wp_dev = np.zeros_like(Wpt)
            Wm_dev[..., :-1] = Wmt[..., 1:]
            Wp_dev[..., 1:] = Wpt[..., :-1]
        else:
            # x-orientation (c, h, w); row h=0 / h=127 already zero
            Wm_dev = np.swapaxes(Wmt, -1, -2)
            Wp_dev = np.swapaxes(Wpt, -1, -2)
        mults[:, 768 * s:768 * s + 384] = \
            Wm_dev.astype(np.float32).transpose(1, 0, 2).reshape(128, 384)
        mults[:, 768 * s + 384:768 * (s + 1)] = \
            Wp_dev.astype(np.float32).transpose(1, 0, 2).reshape(128, 384)

    aux = np.zeros((128, 768), dtype=np.float32)
    aux[:, 0:128] = np.eye(128, dtype=np.float32)
    sd = np.zeros((128, 128), np.float32)
    sd[np.arange(127), np.arange(1, 128)] = 1.0   # out[h] = in[h-1]
    aux[:, 128:256] = sd
    aux[:, 256:384] = sd.T
    aux[:, 384:768] = \
        S_run.astype(np.float32).transpose(1, 0, 2).reshape(128, 384)
    return mults, aux


# ---------------- device program ----------------

def build_program(repeat=1, final_mult=True):
    nc = bass.Bass("TRN2", target_bir_lowering=False, debug=False)

    u_in = nc.dram_tensor("u", [128, CW], F32, kind="ExternalInput")
    m_in = nc.dram_tensor("mults", [128, MCOLS], F32, kind="ExternalInput")
    x_in = nc.dram_tensor("aux", [128, 768], F32, kind="ExternalInput")
    o_out = nc.dram_tensor("out", [128, CW], F32, kind="ExternalOutput")

    axis_of = ["x" if s % 2 == 0 else "y" for s in range(NSW)]

    # ---- deterministic count tables ----
    def vcnt(rep, s, c):        # v_sem value after DVE T12 of (rep,s,c)
        return 3 * (NSW * rep + s) + c + 1

    def acnt(rep, s, c):        # a_sem value after ACT copy of (rep,s,c)
        return 3 * (NSW * rep + s) + c + 1

    def accnt(rep, s, c):       # pe_ac_sem after accum group of (rep,s,c)
        return 3 * (NSW * rep + s) + c + 1

    def shcnt(rep, s, c):       # pe_sh_sem after SU matmul of (rep,s,c)
        ys = 10 * rep + (s - 1) // 2
        return 6 * ys + 2 * c + 2

    def prev_sweep(rep, s):
        return (rep, s - 1) if s > 0 else (rep - 1, NSW - 1)

    def prev_y(rep, s):         # y sweep before y-sweep s (odd), or None
        if s >= 3:
            return (rep, s - 2)
        return (rep - 1, NSW - 2) if rep > 0 else None

    with (
        nc.sbuf_tensor([128, MCOLS], F32) as Mt,
        nc.sbuf_tensor([128, CW], F32) as Vt,
        nc.sbuf_tensor([128, 2 * CW], F32) as Tt,     # T1 | T2
        nc.sbuf_tensor([128, 768], F32) as AX,
        nc.psum_tensor([128, CW], F32) as Pout,
        nc.psum_tensor([128, 2048], F32) as Pscr,     # [SD|SU] x parity
        nc.semaphore() as dma_sem,
        nc.semaphore() as v_sem,
        nc.semaphore() as a_sem,
        nc.semaphore() as sh_sem,
        nc.semaphore() as ac_sem,
        nc.Block() as block,
    ):
        ID = AX[:, 0:128]
        SDm = AX[:, 128:256]
        SUm = AX[:, 256:384]

        def r3(ap2):
            return ap2.rearrange("p (r n) -> p r n", r=BL)

        def vch(c):             # V channel c as (p, r, n)
            return r3(Vt[:, W * c:W * (c + 1)])

        def vch_k(c):           # V channel c as (p, k:2 bcast, r, n)
            return vch(c).unsqueeze(1).broadcast_to([128, 2, BL, S])

        def w_k(s, c):          # weights (p, k:2 stride 384, r:0, n)
            base = 768 * s + 128 * c
            v = Mt[:, base:base + 512].rearrange("p (k n) -> p k n", k=4)
            return v[:, ::3, :].unsqueeze(2).broadcast_to([128, 2, BL, S])

        def t12_k(c):           # T12 out (p, k:2 stride CW, r, n)
            v = Tt[:, W * c:W * c + 2048].rearrange(
                "p (k n) -> p k n", k=4)
            return v[:, ::3, :].rearrange("p k (r n) -> p k r n", r=BL)

        def t_ch(which, c):     # T1/T2 channel c as (p, r, n)
            return r3(Tt[:, CW * which + W * c:CW * which + W * (c + 1)])

        def scr_k(c):           # scratch (p, k:2 stride 512, r, n)
            q = c % 2
            return Pscr[:, 1024 * q:1024 * q + 1024].rearrange(
                "p (k r n) -> p k r n", k=2, r=BL)

        def scr_half(c, k):
            q = c % 2
            return Pscr[:, 1024 * q + 512 * k:1024 * q + 512 * (k + 1)]

        def pch(c):             # POUT channel c (p, r, n)
            return r3(Pout[:, W * c:W * (c + 1)])

        @block.vector
        def _(vector):
            for rep in range(repeat):
                for s in range(NSW):
                    ax = axis_of[s]
                    for c in range(C):
                        if rep == 0 and c == 0:
                            vector.wait_ge(dma_sem, 16 * (3 + s))
                        if ax == "x":
                            if (rep, s) != (0, 0):
                                vector.wait_ge(a_sem, acnt(*prev_sweep(rep, s), c))
                            src = vch_k(c)
                        else:
                            vector.wait_ge(sh_sem, shcnt(rep, s, c))
                            src = scr_k(c)
                        nc.vector.tensor_tensor(
                            t12_k(c), w_k(s, c), src, MUL
                        ).then_inc(v_sem, 1)
            if final_mult:
                for c in range(C):
                    vector.wait_ge(a_sem, acnt(repeat - 1, NSW - 1, c))
                    sf = AX[:, 384 + 128 * c:384 + 128 * (c + 1)]
                    nc.vector.tensor_tensor(
                        t_ch(0, c), vch(c),
                        sf.unsqueeze(1).broadcast_to([128, BL, S]), MUL,
                    ).then_inc(v_sem, 1)

        @block.tensor
        def _(tensor):
            tensor.wait_ge(dma_sem, 16 * 3)
            for rep in range(repeat):
                for s in range(NSW):
                    ax = axis_of[s]
                    if ax == "y":
                        for c in range(C):
                            if (rep, s) != (0, 0):
                                tensor.wait_ge(a_sem, acnt(*prev_sweep(rep, s), c))
                            if c == 2:
                                tensor.wait_ge(v_sem, vcnt(rep, s, 0))
                            else:
                                py = prev_y(rep, s)
                                if py is not None:
                                    tensor.wait_ge(
                                        v_sem, vcnt(*py, 2 if c == 0 else 1))
                            nc.tensor.matmul(
                                scr_half(c, 0), SDm, Vt[:, W * c:W * (c + 1)],
                                start=True, stop=True,
                            ).then_inc(sh_sem, 1)
                            nc.tensor.matmul(
                                scr_half(c, 1), SUm, Vt[:, W * c:W * (c + 1)],
                                start=True, stop=True,
                            ).then_inc(sh_sem, 1)
                    for c in range(C):
                        if ax == "x" and (rep, s) != (0, 0):
                            tensor.wait_ge(a_sem, acnt(*prev_sweep(rep, s), c))
                        tensor.wait_ge(v_sem, vcnt(rep, s, c))
                        nc.tensor.matmul(
                            Pout[:, W * c:W * (c + 1)], ID,
                            Vt[:, W * c:W * (c + 1)],
                            start=True, stop=False, skip_group_check=True,
                        )
                        if ax == "x":
                            nc.tensor.matmul(
                                pch(c)[:, :, 1:S], ID,
                                t_ch(0, c)[:, :, 0:S - 1],
                                start=False, stop=False, skip_group_check=True,
                            )
                            nc.tensor.matmul(
                                pch(c)[:, :, 0:S - 1], ID,
                                t_ch(1, c)[:, :, 1:S],
                                start=False, stop=True, skip_group_check=True,
                            ).then_inc(ac_sem, 1)
                        else:
                            nc.tensor.matmul(
                                Pout[:, W * c:W * (c + 1)], ID,
                                Tt[:, W * c:W * (c + 1)],
                                start=False, stop=False, skip_group_check=True,
                            )
                            nc.tensor.matmul(
                                Pout[:, W * c:W * (c + 1)], ID,
                                Tt[:, CW + W * c:CW + W * (c + 1)],
                                start=False, stop=True, skip_group_check=True,
                            ).then_inc(ac_sem, 1)

        @block.scalar
        def _(scalar):
            for rep in range(repeat):
                for s in range(NSW):
                    for c in range(C):
                        scalar.wait_ge(ac_sem, accnt(rep, s, c))
                        nc.scalar.copy(
                            Vt[:, W * c:W * (c + 1)],
                            Pout[:, W * c:W * (c + 1)],
                        ).then_inc(a_sem, 1)

        @block.sync
        def _(sync):
            sync.dma_start(Vt[:], u_in[:]).then_inc(dma_sem, 16)
            sync.dma_start(AX[:], x_in[:]).then_inc(dma_sem, 16)
            sync.dma_start(Mt[:, 0:768], m_in[:, 0:768]).then_inc(dma_sem, 16)
            sync.wait_ge(dma_sem, 16 * 3)
            for s in range(1, NSW):
                sync.dma_start(
                    Mt[:, 768 * s:768 * (s + 1)],
                    m_in[:, 768 * s:768 * (s + 1)],
                ).then_inc(dma_sem, 16)
                sync.wait_ge(dma_sem, 16 * (3 + s))
            vfin = 3 * NSW * repeat
            for c in range(C):
                sync.wait_ge(v_sem, vfin + c + 1)
                sync.dma_start(
                    o_out[:, W * c:W * (c + 1)], Tt[:, W * c:W * (c + 1)]
                ).then_inc(dma_sem, 16)

    return nc


_PROGRAM = None


def _get_program():
    global _PROGRAM
    if _PROGRAM is None:
        _PROGRAM = build_program()
    return _PROGRAM


def pack_u(u_core):
    """(BL,C,S,S) -> (128, C*BL*S) device layout (h, c, b, w)."""
    return np.ascontiguousarray(
        u_core.transpose(2, 1, 0, 3).reshape(128, CW), dtype=np.float32)


def unpack_out(o_core):
    """(128, C*BL*S) -> (BL,C,S,S)."""
    return np.ascontiguousarray(
        o_core.reshape(128, C, BL, S).transpose(2, 1, 0, 3))


def make_in_maps(u, alpha_base, beta_base, alpha_time_coeff, beta_time_coeff):
    mults, aux = _build_packed(alpha_base, beta_base,
                               alpha_time_coeff, beta_time_coeff)
    u = np.ascontiguousarray(u, dtype=np.float32)
    return [
        {"u": pack_u(u[i * BL:(i + 1) * BL]), "mults": mults, "aux": aux}
        for i in range(NCORES)
    ]


def kernel(u, alpha_base, beta_base, alpha_time_coeff, beta_time_coeff,
           **run_kwargs):
    in_maps = make_in_maps(u, alpha_base, beta_base,
                           alpha_time_coeff, beta_time_coeff)
    nc = _get_program()
    res = None
    last_err = None
    for _attempt in range(3):
        try:
            res = run_bass_kernel_spmd(nc, in_maps, list(range(NCORES)),
                                       **run_kwargs)
            break
        except Exception as e:  # transient NRT device wedges; retry
            last_err = e
    if res is None:
        raise last_err
    out = np.concatenate(
        [unpack_out(res.results[i]["out"]) for i in range(NCORES)], axis=0)
    return np.ascontiguousarray(out, dtype=np.float32)
